# revision 2
# baseline (speedup 1.0000x reference)
"""GQA attention kernel for 8 Trainium2 NeuronCores.

Sharding: batch x head-group. Core c handles batch b = c // 4 and head
group g = c % 4 (8 q heads 8g..8g+7, kv heads 2g, 2g+1). Each core
computes a partial output  attn_out_g[b] @ w_out[rows of g]  and the
host sums the 4 partials per batch.

Single-scope pipelined structure: per token-slab s the emission order is
proj(s) -> attention(q-slab s) -> outproj(s-1), all sharing one PSUM
budget (scores 4 banks, PV 2, proj 1, outproj 1) so the Tile scheduler
can fill tensor-engine idle time during the ACT(exp)-bound attention
stretches with projection and out-projection matmuls.

On-chip layout is fully transposed: x^T is pre-transposed on the host,
q^T/k^T come straight out of the QKV^T projection, V is projected
directly in natural [token, hd] layout, scores are computed as
S^T = K @ Q^T (softmax over the partition dim, denominator via an
appended ones-column in V), and the PV output^T feeds the
out-projection as lhsT. Causal structure trims the diagonal k-tiles to
their live q-range (width 512-128r).
"""

import numpy as np
import ml_dtypes

B, T, D = 2, 2048, 2048
H, KVH, HD = 32, 8, 64
KVD = KVH * HD  # 512
NCORES = 8
SCALE = 1.0 / np.sqrt(HD)

_CACHE = {}


def _build():
    import concourse.bass as bass
    import concourse.mybir as mybir
    import concourse.tile as tile
    from concourse import bacc

    f32 = mybir.dt.float32
    bf16 = mybir.dt.bfloat16
    AF = mybir.ActivationFunctionType
    OP = mybir.AluOpType

    nc = bacc.Bacc("TRN2", target_bir_lowering=False, debug=False)

    xbT = nc.dram_tensor("xbT", [D, T], bf16, kind="ExternalInput")
    wqk = nc.dram_tensor("wqk", [D, 640], bf16, kind="ExternalInput")
    wv = nc.dram_tensor("wv", [D, 128], bf16, kind="ExternalInput")
    wo = nc.dram_tensor("wo", [512, D], bf16, kind="ExternalInput")
    sinT = nc.dram_tensor("sinT", [128, T], bf16, kind="ExternalInput")
    cosT = nc.dram_tensor("cosT", [128, T], bf16, kind="ExternalInput")
    perm = nc.dram_tensor("perm", [128, 128], bf16, kind="ExternalInput")
    ident = nc.dram_tensor("ident", [64, 64], bf16, kind="ExternalInput")
    tri = nc.dram_tensor("tri", [128, 128], bf16, kind="ExternalInput")
    outp = nc.dram_tensor("outp", [T, D], bf16, kind="ExternalOutput")

    DT = D // 128   # 16 d-tiles
    NSLAB = 4       # token slabs of 512
    SLAB = 512

    with tile.TileContext(nc) as tc:
        with (
            tc.tile_pool(name="const", bufs=1) as cpool,
            tc.tile_pool(name="resid", bufs=1) as rpool,
            tc.tile_pool(name="xin", bufs=2) as xpool,
            tc.tile_pool(name="rope", bufs=2) as rpool2,
            tc.tile_pool(name="probs", bufs=3) as ppool,
            tc.tile_pool(name="nrm", bufs=2) as npool,
            tc.tile_pool(name="ost", bufs=2) as opool,
            tc.tile_pool(name="ps_sc", bufs=2, space="PSUM") as ps_sc,
            tc.tile_pool(name="ps_pv", bufs=1, space="PSUM") as ps_pv,
            tc.tile_pool(name="ps_pj", bufs=1, space="PSUM") as ps_pj,
            tc.tile_pool(name="ps_o", bufs=1, space="PSUM") as ps_o,
        ):
            # ---- resident constants ----
            wqk_sb = [cpool.tile([128, 640], bf16, tag=f"wqk{i}", name=f"wqk{i}") for i in range(DT)]
            wv_sb = [cpool.tile([128, 128], bf16, tag=f"wv{i}", name=f"wv{i}") for i in range(DT)]
            wo_sb = [cpool.tile([128, D], bf16, tag=f"wo{i}", name=f"wo{i}") for i in range(4)]
            sin_sb = cpool.tile([128, T], bf16, tag="sin")
            cos_sb = cpool.tile([128, T], bf16, tag="cos")
            perm_sb = cpool.tile([128, 128], bf16, tag="perm")
            ident_sb = cpool.tile([64, 64], bf16, tag="ident")
            tri_sb = cpool.tile([128, 128], bf16, tag="tri")
            ones_sb = cpool.tile([65, 64], bf16, tag="ones")

            # persistent activations
            qkT = [rpool.tile([128, T], bf16, tag=f"qkT{e}", name=f"qkT{e}") for e in range(5)]
            vnat = [rpool.tile([128, 130], bf16, tag=f"vn{k}", name=f"vn{k}") for k in range(16)]
            attnT = [rpool.tile([128, T], bf16, tag=f"attnT{j}", name=f"attnT{j}") for j in range(4)]

            wqk3 = wqk.rearrange("(o p) e -> p o e", p=128)
            wv3 = wv.rearrange("(o p) e -> p o e", p=128)
            wo3 = wo.rearrange("(o p) e -> p o e", p=128)

            # slab-0 x tiles interleaved with the weights they pair with
            xT = [xpool.tile([128, SLAB], bf16, tag=f"xT{d}", name=f"xT0_{d}") for d in range(DT)]
            for d in range(DT):
                nc.sync.dma_start(wqk_sb[d][:], wqk3[:, d])
                nc.sync.dma_start(xT[d][:], xbT[d * 128:(d + 1) * 128, 0:SLAB])
            for d in range(DT):
                nc.sync.dma_start(wv_sb[d][:], wv3[:, d])
            nc.sync.dma_start(sin_sb[:], sinT[:])
            nc.sync.dma_start(cos_sb[:], cosT[:])
            nc.sync.dma_start(perm_sb[:], perm[:])
            nc.sync.dma_start(ident_sb[:], ident[:])
            nc.sync.dma_start(tri_sb[:], tri[:])
            for i in range(4):
                nc.sync.dma_start(wo_sb[i][:], wo3[:, i])
            nc.gpsimd.memset(ones_sb[:], 1.0)
            for k in range(16):
                nc.gpsimd.memset(vnat[k][:], 1.0)

            def emit_proj(s, xT):
                sl = slice(s * SLAB, (s + 1) * SLAB)
                # q/k transposed projection + rope
                for e in range(5):
                    acc = ps_pj.tile([128, SLAB], f32, tag="ps")
                    for d in range(DT):
                        nc.tensor.matmul(
                            acc[:], wqk_sb[d][:, e * 128:(e + 1) * 128], xT[d][:],
                            start=(d == 0), stop=(d == DT - 1),
                        )
                    raw = rpool2.tile([128, SLAB], bf16, tag="raw")
                    nc.scalar.copy(raw[:], acc[:])
                    rot = ps_pj.tile([128, SLAB], f32, tag="ps")
                    nc.tensor.matmul(rot[:], perm_sb[:], raw[:], start=True, stop=True)
                    m2 = rpool2.tile([128, SLAB], bf16, tag="m2")
                    nc.vector.tensor_tensor(m2[:], raw[:], cos_sb[:, sl], OP.mult)
                    m1 = rpool2.tile([128, SLAB], bf16, tag="m1")
                    nc.vector.tensor_tensor(m1[:], rot[:], sin_sb[:, sl], OP.mult)
                    nc.vector.tensor_tensor(qkT[e][:, sl], m1[:], m2[:], OP.add)
                # v in natural [token, hd] layout: [vA | 1 | vB | 1]
                for t in range(4):
                    kt = 4 * s + t
                    vp = ps_pj.tile([128, SLAB], f32, tag="ps")
                    for d in range(DT):
                        nc.tensor.matmul(
                            vp[:, 0:128], xT[d][:, t * 128:(t + 1) * 128], wv_sb[d][:],
                            start=(d == 0), stop=(d == DT - 1),
                        )
                    nc.vector.tensor_copy(vnat[kt][:, 0:64], vp[:, 0:64])
                    nc.vector.tensor_copy(vnat[kt][:, 65:129], vp[:, 64:128])

            def emit_attn(s):
                qhi = (s + 1) * SLAB
                nkt = 4 * s + 4
                for j in range(4):
                    pvA = ps_pv.tile([65, SLAB], f32, tag="pvA")
                    pvB = ps_pv.tile([65, SLAB], f32, tag="pvB")
                    for kt in range(nkt):
                        r = kt - 4 * s
                        w = SLAB if r < 0 else SLAB - 128 * r
                        offB = SLAB if (r < 0 or r == 1) else w
                        c0 = SLAB - w
                        ksl = slice(kt * 128, (kt + 1) * 128)
                        qsl = slice(s * SLAB + c0, qhi)
                        sc = ps_sc.tile([128, 1024], f32, tag="sc")
                        nc.tensor.matmul(
                            sc[:, 0:w], qkT[4][0:64, ksl], qkT[j][0:64, qsl],
                            start=True, stop=True)
                        nc.tensor.matmul(
                            sc[:, offB:offB + w], qkT[4][64:128, ksl], qkT[j][64:128, qsl],
                            start=True, stop=True)
                        p = ppool.tile([128, 1024], bf16, tag="p")
                        if offB == w:
                            nc.scalar.activation(
                                p[:, 0:2 * w], sc[:, 0:2 * w], AF.Exp, scale=float(SCALE))
                        else:
                            nc.scalar.activation(
                                p[:, 0:w], sc[:, 0:w], AF.Exp, scale=float(SCALE))
                            nc.scalar.activation(
                                p[:, offB:offB + w], sc[:, offB:offB + w],
                                AF.Exp, scale=float(SCALE))
                        if r >= 0:
                            nc.vector.tensor_tensor(
                                p[:, 0:128], p[:, 0:128], tri_sb[:], OP.mult)
                            nc.vector.tensor_tensor(
                                p[:, offB:offB + 128], p[:, offB:offB + 128],
                                tri_sb[:], OP.mult)
                        nc.tensor.matmul(
                            pvA[0:65, c0:SLAB], vnat[kt][:, 0:65], p[:, 0:w],
                            start=(kt == 0), stop=(kt == nkt - 1),
                        )
                        nc.tensor.matmul(
                            pvB[0:65, c0:SLAB], vnat[kt][:, 65:130], p[:, offB:offB + w],
                            start=(kt == 0), stop=(kt == nkt - 1),
                        )
                    # normalize: den rows -> broadcast -> reciprocal -> scale
                    qsl2 = slice(s * SLAB, qhi)
                    den = npool.tile([65, 1024], bf16, tag="den")
                    nc.vector.tensor_copy(den[64:65, 0:512], pvA[64:65, :])
                    nc.vector.tensor_copy(den[64:65, 512:1024], pvB[64:65, :])
                    stgA = npool.tile([64, SLAB], bf16, tag="stgA")
                    nc.vector.tensor_copy(stgA[:], pvA[0:64])
                    stgB = npool.tile([64, SLAB], bf16, tag="stgB")
                    nc.vector.tensor_copy(stgB[:], pvB[0:64])
                    tn = ps_sc.tile([128, 1024], f32, tag="sc")
                    nc.tensor.matmul(
                        tn[0:64, 0:512], ones_sb[64:65, :], den[64:65, 0:512],
                        start=True, stop=True)
                    nc.tensor.matmul(
                        tn[64:128, 0:512], ones_sb[64:65, :], den[64:65, 512:1024],
                        start=True, stop=True)
                    rec = npool.tile([128, SLAB], f32, tag="rec")
                    nc.vector.reciprocal(rec[:], tn[:, 0:512])
                    nc.vector.tensor_tensor(
                        attnT[j][0:64, qsl2], stgA[:], rec[0:64], OP.mult)
                    nc.tensor.matmul(
                        tn[64:128, 512:1024], ident_sb[:], stgB[:], start=True, stop=True)
                    nc.vector.tensor_tensor(
                        attnT[j][64:128, qsl2], tn[64:128, 512:1024], rec[64:128], OP.mult)

            def emit_outproj(s):
                for i in range(4 * s, 4 * s + 4):
                    isl = slice(i * 128, (i + 1) * 128)
                    for ns in range(4):
                        nsl = slice(ns * SLAB, (ns + 1) * SLAB)
                        po = ps_o.tile([128, SLAB], f32, tag="po")
                        for j in range(4):
                            nc.tensor.matmul(
                                po[:], attnT[j][:, isl], wo_sb[j][:, nsl],
                                start=(j == 0), stop=(j == 3),
                            )
                        ot = opool.tile([128, SLAB], bf16, tag="ot")
                        nc.vector.tensor_copy(ot[:], po[:])
                        nc.sync.dma_start(outp[isl, nsl], ot[:])

            for s in range(NSLAB):
                emit_proj(s, xT)
                if s + 1 < NSLAB:
                    xT = [xpool.tile([128, SLAB], bf16, tag=f"xT{d}", name=f"xT{s+1}_{d}")
                          for d in range(DT)]
                    for d in range(DT):
                        nc.sync.dma_start(
                            xT[d][:], xbT[d * 128:(d + 1) * 128, (s + 1) * SLAB:(s + 2) * SLAB])
                emit_attn(s)
                if s >= 1:
                    emit_outproj(s - 1)
            emit_outproj(3)

    nc.finalize()
    return nc


def _host_inputs(x, sin, cos, w_qkv, w_out):
    bf = ml_dtypes.bfloat16
    sinT_np = np.concatenate([sin.T, sin.T], axis=0).astype(bf)  # [128, T]
    cosT_np = np.concatenate([cos.T, cos.T], axis=0).astype(bf)

    perm_np = np.zeros((128, 128), np.float32)
    for blk in range(2):
        for p in range(64):
            k = blk * 64 + ((p + 32) % 64)
            perm_np[k, blk * 64 + p] = -1.0 if p < 32 else 1.0
    perm_np = perm_np.astype(bf)
    ident_np = np.eye(64, dtype=np.float32).astype(bf)
    tri_np = np.triu(np.ones((128, 128), np.float32)).astype(bf)

    xbT_np = [np.ascontiguousarray(x[b].T).astype(bf) for b in range(B)]

    in_maps = []
    for c in range(NCORES):
        b, g = divmod(c, 4)
        cols = []
        for j in range(4):
            h1, h2 = 8 * g + j, 8 * g + 4 + j
            cols.append(w_qkv[:, 64 * h1:64 * h1 + 64])
            cols.append(w_qkv[:, 64 * h2:64 * h2 + 64])
        cols.append(w_qkv[:, D + 128 * g: D + 128 * g + 128])  # k heads 2g,2g+1
        wqk_np = np.concatenate(cols, axis=1).astype(bf)
        wv_np = w_qkv[:, D + KVD + 128 * g: D + KVD + 128 * g + 128].astype(bf)
        rows = []
        for j in range(4):
            h1, h2 = 8 * g + j, 8 * g + 4 + j
            rows.append(w_out[64 * h1:64 * h1 + 64, :])
            rows.append(w_out[64 * h2:64 * h2 + 64, :])
        wo_np = np.concatenate(rows, axis=0).astype(bf)
        in_maps.append({
            "xbT": xbT_np[b],
            "wqk": wqk_np,
            "wv": wv_np,
            "wo": wo_np,
            "sinT": sinT_np,
            "cosT": cosT_np,
            "perm": perm_np,
            "ident": ident_np,
            "tri": tri_np,
        })
    return in_maps


def kernel(x, sin, cos, w_qkv, w_out, _trace=False):
    from concourse.bass_utils import run_bass_kernel_spmd

    if "nc" not in _CACHE:
        _CACHE["nc"] = _build()
    nc = _CACHE["nc"]

    in_maps = _host_inputs(
        np.asarray(x), np.asarray(sin), np.asarray(cos),
        np.asarray(w_qkv), np.asarray(w_out))
    res = run_bass_kernel_spmd(
        nc, in_maps, core_ids=list(range(NCORES)), trace=_trace)
    out = np.zeros((B, T, D), np.float32)
    for c in range(NCORES):
        b = c // 4
        out[b] += res.results[c]["outp"].astype(np.float32)
    if _trace:
        kernel.last_result = res
    return out
